# revision 1
# baseline (speedup 1.0000x reference)
"""Trainium2 Bass kernel for nn_GAT_12232066859439.

3-layer GAT + 6-head MLP readout, ~133-153us HW (baseline 192-205us).

  - GAT replicated on all 8 cores. The scrambled-view attention collapses to
    att[i,j] = adj[i,j]*c[j, i//F] / rowsum with c = exp(lrelu(e1[j] +
    e2T[j%F, u])); e1 rides as an extra column of the projection matmul
    (precomposed W @ a[:F]); e2's F=64 partition-dup is a second matmul
    group at partition offset 64 (a sbuf-sbuf DMA would queue ~11us behind
    the bulk A8 stream).
    Layer 1 (F=128): haug f-major, M2_u = haug * c with inner-stride-1 APs.
    Layers 2/3 (F=64): w = adj*c in the i-major adj layout (elementwise,
    fast mode); per-kc weight slices are single-stride APs.
    Per layer: all scale-muls, then all matmul groups, then all epilogues —
    interleaved emission head-of-line blocks the FIFO engine queues.
    nps psum is copied to SBUF immediately so matmul groups never throttle
    on epilogue drain; the h-bias is applied per-partition after the
    transpose (one DVE op fewer per chunk).
  - The 100MB l1 matvec is sharded 192 rows/core, stored fp8e4 (x32) with
    fp8 xf (x256), DoubleRow perf mode (weights need the 3D [Ki,2,dim] AP
    with Ko step % 16 == 0 and a 16-bit step field: xf pairs sit 16 cols
    apart, A8 tile order is host-permuted to keep the moving s-stride at
    384). Layer-3 output writes straight into xf (no transpose); the matvec
    runs as one warm PE stream right behind the layer-3 matmuls (163ns per
    out-[2,384] step, the DoubleRow floor).
  - Each core computes its partial l2 head contraction (block-diagonal
    [192,768] slice of l2w); one AllReduce of [1,768] replaces the old
    AllGather + on-core l2; sigmoid/l3 tail runs as [6,128] on all cores.
  - All constants ride in a few packed fan-out DMAs; xT is kc-major so the
    first hproj only needs the first 1028 const columns.
  - A dummy AllGather gated on the last xf chunk warms the CC and absorbs
    the cross-core launch skew during the matvec, so the real AllReduce
    starts warm (~1us trigger-to-mesh instead of ~11.5us cold) with a
    short rendezvous; l2b/8 is folded into each core's z2 partials.
  Typical HW time 124-138us; residual variance is launch-skew absorbed at
  the dummy rendezvous plus ~9us of end-of-kernel semaphore teardown.
"""
import os
import sys

sys.path.insert(0, "/opt/trn_rl_repo")

import numpy as np

import concourse.bacc as bacc
import concourse.bass as bass
import concourse.tile as tile
from concourse import mybir
from concourse.bass_utils import run_bass_kernel_spmd

F32 = mybir.dt.float32
F16 = mybir.dt.float16
F8 = mybir.dt.float8e4
AF = mybir.ActivationFunctionType
ALU = mybir.AluOpType
PM = mybir.MatmulPerfMode

P = 128
N = 1024
NCORES = 8
NCH = N // P
LAYERS = [(512, 128, 8), (128, 64, 16), (64, 64, 16)]
RSHARD = 1536 // NCORES      # 192 l1 rows per core
KCH = 65536 // P             # 512 k-tiles for the matvec
SA = 32.0                    # A fp8 scale
SX = 256.0                   # xf fp8 scale
WARM_COLLECTIVE = int(os.environ.get("GAT_WARM_COLL", "0"))

# C16 packed fp16 const layout (columns); Waug1+xT first so the first
# hproj can start as soon as the first const DMA lands
C16_WAUG1 = 0                # [P, 4*129]
C16_XT = 516                 # [P, 4*1024]
C16_WAUG2 = 4612             # [P, 65]
C16_WAUG3 = 4677             # [64, 65]
C16_WSEL1 = 4742             # [P, 8*8]
C16_WSEL2 = 4806             # [P, 8*16]
C16_WSEL3 = 4934             # [P, 8*16]
C16_IDT = 5062               # [P, 128] fp16 identity
C16_COLS = 5062 + 128

# C32 packed fp32 const layout
C32_IDENT = 0                # [P, 128]
C32_B1 = 128                 # [P, 128]
C32_B2 = 256                 # [P, 64]
C32_B3 = 320                 # [P, 64]
C32_BT = 384                 # [P, 3] per-feature biases (post-transpose)
C32_COLS = 388

_CACHE = {}


def ts(i, n):
    return slice(i * n, (i + 1) * n)


def _build():
    nc = bacc.Bacc("TRN2", target_bir_lowering=False, debug=False,
                   num_devices=NCORES)

    c16_d = nc.dram_tensor("C16", [P, C16_COLS], F16, kind="ExternalInput")
    adj_d = nc.dram_tensor("ADJT", [P, N * NCH], F16, kind="ExternalInput")
    c32_d = nc.dram_tensor("C32", [P, C32_COLS], F32, kind="ExternalInput")
    c32r_d = nc.dram_tensor("C32R", [6, 257], F32, kind="ExternalInput")
    a8_d = nc.dram_tensor("A8", [P, KCH, RSHARD], F8, kind="ExternalInput")
    l2s_d = nc.dram_tensor("L2S", [P, 2 * 768], F16, kind="ExternalInput")
    lb_d = nc.dram_tensor("LB", [P, 2], F32, kind="ExternalInput")
    l2b8_d = nc.dram_tensor("L2B8", [1, 768], F32, kind="ExternalInput")
    out_d = nc.dram_tensor("out", [6, 1], F32, kind="ExternalOutput")

    with tile.TileContext(nc) as tc:
        with tc.tile_pool(name="const", bufs=1) as const, \
             tc.tile_pool(name="work", bufs=1) as work, \
             tc.tile_pool(name="pmv", bufs=1, space="PSUM") as pmv, \
             tc.tile_pool(name="dram", bufs=1, space="DRAM") as dram:

            if WARM_COLLECTIVE:
                # 1 = full-group warmup (syncs cores, can stall via sem
                # reuse); 2 = per-core self-groups (no rendezvous)
                groups = ([list(range(NCORES))] if WARM_COLLECTIVE == 1
                          else [[c] for c in range(NCORES)])
                wsb = const.tile([1, 8], F32, name="wsb")
                nc.vector.memset(wsb[:], 1.0)
                w_in = dram.tile([1, 8], F32, name="w_in")
                w_out = dram.tile([1, 8], F32, name="w_out",
                                  addr_space="Shared")
                nc.sync.dma_start(w_in[:], wsb[:])
                nc.gpsimd.collective_compute(
                    "AllReduce", ALU.add,
                    replica_groups=groups,
                    ins=[w_in.opt()], outs=[w_out.opt()])

            # ---- packed constant loads (each dma_start fans out over all
            # DMA queues; trigger issue on sync costs ~0.65us apiece) ----
            c16 = const.tile([P, C16_COLS], F16, name="c16")
            nc.sync.dma_start(c16[:, 0:1028], c16_d[:, 0:1028])
            nc.sync.dma_start(c16[:, 1028:C16_WAUG2], c16_d[:, 1028:C16_WAUG2])
            nc.sync.dma_start(c16[:, C16_WAUG2:], c16_d[:, C16_WAUG2:])
            c32 = const.tile([P, C32_COLS], F32, name="c32")
            nc.sync.dma_start(c32[:], c32_d[:])
            adjt = const.tile([P, N * NCH], F16, name="adjt")
            for i in range(2):
                nc.sync.dma_start(adjt[:, ts(i, 4 * N)],
                                  adj_d[:, ts(i, 4 * N)])
            c32r = const.tile([6, 257], F32, name="c32r")
            nc.sync.dma_start(c32r[:], c32r_d[:])
            l2s = const.tile([P, 2 * 768], F16, name="l2s")
            nc.sync.dma_start(l2s[:], l2s_d[:])
            lb = const.tile([P, 2], F32, name="lb")
            nc.sync.dma_start(lb[:], lb_d[:])
            l2b8 = const.tile([1, 768], F32, name="l2b8")
            nc.sync.dma_start(l2b8[:], l2b8_d[:])
            a8 = const.tile([P, KCH, RSHARD], F8, name="a8")
            for s in range(4):
                nc.sync.dma_start(a8[:, ts(s, 128), :], a8_d[:, ts(s, 128), :])

            ident = c32[:, C32_IDENT:C32_IDENT + 128]
            idt = c16[:, C16_IDT:C16_IDT + 128]
            waug = [c16[:, C16_WAUG1:C16_WAUG1 + 516],
                    c16[:, C16_WAUG2:C16_WAUG2 + 65],
                    c16[0:64, C16_WAUG3:C16_WAUG3 + 65]]
            wsel = [c16[:, C16_WSEL1:C16_WSEL1 + 64],
                    c16[:, C16_WSEL2:C16_WSEL2 + 128],
                    c16[:, C16_WSEL3:C16_WSEL3 + 128]]
            bT = c32[:, C32_BT:C32_BT + 3]
            bb = [c32[:, C32_B1:C32_B1 + 128],
                  c32[:, C32_B2:C32_B2 + 64],
                  c32[:, C32_B3:C32_B3 + 64]]
            xT = c16[:, C16_XT:C16_XT + 4 * N]
            adjv = adjt.rearrange("p (i k) -> p i k", k=NCH)  # adj i-major

            xf3 = work.tile([P, KCH], F8, name="xf3")
            xfv = xf3.rearrange("p (q s w) -> p q s w", s=2, w=16)
            a8v = a8.rearrange("p (v s j) r -> p v s (j r)", s=2, j=2)
            t1ps = pmv.tile([2, 2 * RSHARD], F32, name="t1ps")
            hT = None

            # ---- GAT layers ----
            with tc.tile_pool(name="psg", bufs=1, space="PSUM") as psg:
                for l, (Fin, F, g) in enumerate(LAYERS):
                    nk = max(1, Fin // P)
                    kp = min(P, Fin)
                    FA = F + 1  # h features + ones column (kc-major blocks)

                    # layer 1 haug is f-major (fast M2 muls); layers 2/3 are
                    # kc-major (fast hproj copies, w rides on the adj side)
                    haug = work.tile([P, FA * NCH], F16, name=f"haug{l}")
                    hfv = haug.rearrange("p (f k) -> p f k", k=NCH)
                    e1 = work.tile([P, NCH], F32, name=f"e1_{l}")
                    if l == 0:
                        nc.vector.memset(haug[:, F * NCH:FA * NCH], 1.0)
                    else:
                        nc.vector.memset(
                            haug.rearrange("p (k f) -> p k f",
                                           f=FA)[:, :, F], 1.0)

                    for kc in range(NCH):
                        hp = psg.tile([P, 129], F32, name=f"hp{l}_{kc}",
                                      tag="ps129", bufs=5)
                        for ks in range(nk):
                            if l == 0:
                                lhsT = xT[:, kc * 512 + ks * P:
                                          kc * 512 + (ks + 1) * P]
                                rhs = waug[0][:, ts(ks, 129)]
                            else:
                                lhsT = hT[0:kp, ts(kc, P)]
                                rhs = waug[l]
                            nc.tensor.matmul(hp[:, 0:FA], lhsT, rhs,
                                             start=(ks == 0),
                                             stop=(ks == nk - 1))
                        if l == 0:
                            nc.vector.tensor_copy(hfv[:, 0:F, kc],
                                                  hp[:, 0:F])
                        else:
                            nc.scalar.activation(haug[:, kc * FA:kc * FA + F],
                                                 hp[:, 0:F], AF.Copy)
                        nc.scalar.activation(e1[:, kc:kc + 1], hp[:, F:F + 1],
                                             AF.Copy)

                    # e2T[v, u] (v = j mod F); for F=64 write the upper
                    # partition half with a second matmul group (a sbuf-sbuf
                    # dup DMA would queue behind the bulk A8 stream)
                    e2p = psg.tile([P, 129], F32, name=f"e2p{l}", tag="ps129",
                                   bufs=5)
                    for m in range(NCH):
                        lhsT2 = (hfv[:, 0:F, m] if l == 0
                                 else haug[:, m * FA:m * FA + F])
                        nc.tensor.matmul(e2p[0:F, 0:g], lhsT2,
                                         wsel[l][:, ts(m, g)],
                                         start=(m == 0), stop=(m == NCH - 1))
                    if F == 64:
                        for m in range(NCH):
                            lhsT2 = haug[:, m * FA:m * FA + F]
                            nc.tensor.matmul(e2p[64:128, 0:g], lhsT2,
                                             wsel[l][:, ts(m, g)],
                                             start=(m == 0),
                                             stop=(m == NCH - 1))

                    # c[j, u] = exp(lrelu(e1[j] + e2T[j%F, u])), u-major
                    s_scr = work.tile([P, g * NCH], F32, name=f"sscr{l}")
                    sv = s_scr.rearrange("p (u k) -> p u k", k=NCH)
                    nc.vector.tensor_add(
                        sv, e1[:].broadcast_to([P, NCH, g]).rearrange(
                            "p k u -> p u k"),
                        e2p[:, 0:g].broadcast_to([P, g, NCH]))
                    nc.vector.scalar_tensor_tensor(s_scr[:], s_scr[:], 0.2,
                                                   s_scr[:], ALU.mult,
                                                   ALU.max)
                    e_all = work.tile([P, g * NCH], F16, name=f"eall{l}")
                    nc.scalar.activation(e_all[:], s_scr[:], AF.Exp)

                    # phase A: all scale-muls, fast-mode APs (inner stride 1
                    # on every operand), emitted before any epilogue ops so
                    # the DVE queue never head-of-line blocks
                    sc = []
                    for q in range(NCH):
                        M2 = work.tile([P, 1032], F16,
                                       name=f"M2_{l}_{q}", tag="M2", bufs=8)
                        if l == 0:
                            # M2_u[p, f*8+k] = haug[p, f, k] * c[p-as-j, u]
                            nc.vector.tensor_mul(
                                M2[:, 0:FA * NCH].rearrange(
                                    "p (f k) -> p f k", k=NCH),
                                hfv,
                                e_all[:, ts(q, NCH)].broadcast_to(
                                    [P, NCH, FA]).rearrange("p k f -> p f k"))
                        else:
                            # w2[p, i_loc*8 + k] = adj[i, j]*c[j, 2q+h(i)]
                            # (same i-major layout as adjt; i_loc = h*64+i2)
                            nc.vector.tensor_mul(
                                M2[:, 0:NCH * P].rearrange(
                                    "p (h i k) -> p h i k", h=2, k=NCH),
                                adjt[:, q * NCH * P:(q + 1) * NCH * P]
                                .rearrange("p (h i k) -> p h i k",
                                           h=2, k=NCH),
                                e_all[:, ts(q, 16)].rearrange(
                                    "p (h k) -> p h k", k=NCH).broadcast_to(
                                    [P, 2, NCH, 64]).rearrange(
                                    "p h k i -> p h i k"))
                        sc.append(M2)

                    # phase B/C: matmul groups + per-chunk epilogues
                    if l == 1:
                        hTn = work.tile([P, N], F16, name="hT1")
                    elif l == 0:
                        hT = work.tile([P, N], F16, name="hT0")
                    # phase B: every att matmul group back-to-back on the
                    # PE queue (epilogue PE ops would head-of-line block
                    # later groups otherwise, and the PE never warms)
                    npss = []
                    for q in range(NCH):
                        nps = psg.tile([P, 129], F32, name=f"nps{l}_{q}",
                                       tag="ps129", bufs=5)
                        M2v = (None if l == 0 else
                               sc[q][:, 0:NCH * P].rearrange(
                                   "p (i k) -> p i k", k=NCH))
                        for kc in range(NCH):
                            if l == 0:
                                nc.tensor.matmul(
                                    nps[:, 0:FA], adjv[:, ts(q, P), kc],
                                    sc[q][:, 0:FA * NCH].rearrange(
                                        "p (f k) -> p f k",
                                        k=NCH)[:, :, kc],
                                    start=(kc == 0), stop=(kc == NCH - 1))
                            else:
                                nc.tensor.matmul(
                                    nps[:, 0:FA], M2v[:, :, kc],
                                    haug[:, ts(kc, FA)],
                                    start=(kc == 0), stop=(kc == NCH - 1))
                        # free the psum slot right away so later matmul
                        # groups never throttle on epilogue drain
                        nsb = work.tile([P, 132], F32, name=f"nsb{l}_{q}",
                                        tag="nsb", bufs=8)
                        nc.vector.tensor_copy(nsb[:, 0:FA], nps[:, 0:FA])
                        npss.append(nsb)
                    # phase C: epilogues (from SBUF copies)
                    for q in range(NCH):
                        nps = npss[q]
                        rd = work.tile([P, 1], F32, name=f"rd{l}_{q}",
                                       tag="rd", bufs=4)
                        nc.vector.reciprocal(rd[:], nps[:, F:F + 1])
                        y = work.tile([P, P], F16, name=f"y{l}_{q}",
                                      tag="y", bufs=4)
                        nc.scalar.activation(y[:, 0:F], nps[:, 0:F], AF.Relu,
                                             scale=rd[:])
                        if l == 0:
                            # bias rides per-partition after the transpose
                            tp = psg.tile([P, P], F16, name=f"tp{l}_{q}",
                                          tag="tp", bufs=2)
                            nc.tensor.transpose(tp[:], y[:], idt)
                            nc.scalar.activation(hT[:, ts(q, P)], tp[:],
                                                 AF.Relu, bias=bT[:, 0:1])
                        elif l == 1:
                            tp = psg.tile([P, P], F16, name=f"tp{l}_{q}",
                                          tag="tp", bufs=2)
                            nc.tensor.transpose(tp[0:64, :], y[:, 0:64],
                                                idt)
                            nc.scalar.activation(hTn[0:64, ts(q, P)],
                                                 tp[0:64, :], AF.Relu,
                                                 bias=bT[0:64, 1:2])
                        else:
                            hn = work.tile([P, P], F16, name=f"hn{l}_{q}",
                                           tag="hn", bufs=4)
                            nc.vector.tensor_add(hn[:, 0:F], y[:, 0:F],
                                                 bb[l])
                            nc.scalar.activation(xf3[:, ts(q, 64)],
                                                 hn[:, 0:64], AF.Relu,
                                                 scale=SX)
                    if l == 2:
                        # matvec after all epilogues are queued: step V eats
                        # xf cols {c0, c0+1, c0+16, c0+17} with
                        # c0 = 32*(V//8) + 2*(V%8); runs as one warm PE
                        # stream right behind the layer-3 matmuls
                        for V in range(128):
                            nc.tensor.matmul(
                                t1ps[:],
                                xfv[:, V // 8, :,
                                    2 * (V % 8):2 * (V % 8) + 2],
                                a8v[:, V, :, :],
                                start=(V == 0), stop=(V == 127),
                                perf_mode=PM.DoubleRow)
                        # warm the CC during the matvec: dummy collective
                        # whose input DMA waits for the last xf chunk, so
                        # the trigger fires here (not at kernel start) and
                        # the real AllReduce starts warm and pre-synced
                        w8i = dram.tile([1, 8], F8, name="w8i")
                        nc.sync.dma_start(w8i[:], xf3[0:1, 448:456])
                        w8o = dram.tile([8, 8], F8, name="w8o",
                                        addr_space="Shared")
                        nc.gpsimd.collective_compute(
                            "AllGather", ALU.bypass,
                            replica_groups=[list(range(NCORES))],
                            ins=[w8i.opt()], outs=[w8o.opt()])
                    if l == 1:
                        hT = hTn

            # hoist activation-table loads into the matvec window
            dact = work.tile([1, 2], F32, name="dact")
            nc.scalar.activation(dact[:, 0:1], xf3[:1, 0:1], AF.Sigmoid)
            nc.scalar.activation(dact[:, 1:2], xf3[:1, 0:1], AF.Relu,
                                 bias=lb[0:1, 0:1])

            with tc.tile_pool(name="pst", bufs=1, space="PSUM") as pst:
                # t1 = relu(z/(SA*SX) + b): transpose both psum rows in
                # column pairs (no cross-partition DMA), add on partitions
                t1c = work.tile([2, 2 * RSHARD], F32, name="t1c")
                nc.vector.tensor_copy(t1c[:], t1ps[:])
                tps = []
                for i, (c0, c1) in enumerate(((0, 128), (192, 320),
                                              (128, 192), (320, 384))):
                    tt = pst.tile([P, 2], F32, name=f"tt{i}", tag="tt",
                                  bufs=4)
                    nc.tensor.transpose(tt[0:c1 - c0, :], t1c[:, c0:c1],
                                        ident[0:2, 0:2])
                    tps.append(tt)
                sb1 = work.tile([P, 2], F32, name="sb1")
                nc.scalar.activation(sb1[:, 0:1], tps[1][:, 1:2], AF.Copy)
                nc.scalar.activation(sb1[0:64, 1:2], tps[3][0:64, 1:2],
                                     AF.Copy)
                za = work.tile([P, 1], F32, name="za")
                nc.vector.tensor_add(za[:], tps[0][:, 0:1], sb1[:, 0:1])
                zb = work.tile([64, 1], F32, name="zb")
                nc.vector.tensor_add(zb[:], tps[2][0:64, 0:1],
                                     sb1[0:64, 1:2])
                t1sa = work.tile([P, 1], F16, name="t1sa")
                nc.scalar.activation(t1sa[:], za[:], AF.Relu,
                                     scale=1.0 / (SA * SX), bias=lb[:, 0:1])
                t1sb = work.tile([64, 1], F16, name="t1sb")
                nc.scalar.activation(t1sb[:], zb[:], AF.Relu,
                                     scale=1.0 / (SA * SX), bias=lb[0:64, 1:2])

                z2 = work.tile([1, 768], F32, name="z2")
                for half in range(2):
                    ps2 = pst.tile([1, 384], F32, name=f"ps2_{half}",
                                   tag="ps2", bufs=2)
                    nc.tensor.matmul(ps2[:], t1sa[:],
                                     l2s[:, half * 384:half * 384 + 384],
                                     start=True, stop=False)
                    nc.tensor.matmul(
                        ps2[:], t1sb[:],
                        l2s[0:64, 768 + half * 384:768 + half * 384 + 384],
                        start=False, stop=True)
                    nc.vector.tensor_add(z2[:, ts(half, 384)], ps2[:],
                                         l2b8[:, ts(half, 384)])

                rr_in = dram.tile([1, 768], F32, name="rr_in")
                rr_out = dram.tile([1, 768], F32, name="rr_out",
                                   addr_space="Shared")
                nc.sync.dma_start(rr_in[:], z2[:])
                nc.gpsimd.collective_compute(
                    "AllReduce", ALU.add,
                    replica_groups=[list(range(NCORES))],
                    ins=[rr_in.opt()], outs=[rr_out.opt()])

                # ---- tail on [6, 128]: sigmoid(z+l2b), l3w.t2 + l3b ----
                zz = work.tile([6, P], F32, name="zz")
                nc.sync.dma_start(
                    zz[:], rr_out.rearrange("a (h o) -> (a h) o", o=P))
                t2 = work.tile([6, P], F32, name="t2")
                nc.scalar.activation(t2[:], zz[:], AF.Sigmoid)
                p3 = work.tile([6, P], F32, name="p3")
                nc.vector.tensor_mul(p3[:], t2[:], c32r[:, 128:256])
                o6 = work.tile([6, 1], F32, name="o6")
                nc.vector.reduce_sum(o6[:], p3[:], axis=mybir.AxisListType.X)
                oo = work.tile([6, 1], F32, name="oo")
                nc.vector.tensor_add(oo[:], o6[:], c32r[:, 256:257])
                nc.sync.dma_start(out_d[:], oo[:])

    nc.compile()
    return nc


def _prep_inputs(inputs):
    f8 = mybir.dt.np(F8)
    x = np.asarray(inputs["x"], dtype=np.float32)
    adj = np.asarray(inputs["adj"])

    def chunked(arr, nch):
        # [nch*P, C] -> [P, nch*C] with block kc at cols [kc*C:(kc+1)*C]
        c = arr.shape[1]
        return arr.reshape(nch, P, c).transpose(1, 0, 2).reshape(P, nch * c)

    c16 = np.zeros((P, C16_COLS), dtype=np.float16)
    c32 = np.zeros((P, C32_COLS), dtype=np.float32)
    for l, (Fin, F, g) in enumerate(LAYERS):
        W = np.asarray(inputs[f"W{l+1}"], dtype=np.float64)
        a = np.asarray(inputs[f"a{l+1}"], dtype=np.float64)
        b = np.asarray(inputs[f"b{l+1}"], dtype=np.float32)
        waug = np.concatenate([W, (W @ a[:F])[:, None]], axis=1)  # [Fin,F+1]
        off = [C16_WAUG1, C16_WAUG2, C16_WAUG3][l]
        if l == 0:
            c16[:, off:off + 516] = chunked(waug, 4).astype(np.float16)
        else:
            c16[0:Fin, off:off + F + 1] = waug.astype(np.float16)
        aS = a[F:]
        i = np.arange(N)
        wm = np.zeros((N, g), dtype=np.float64)
        wm[i, i % g] = aS[i // g]
        woff = [C16_WSEL1, C16_WSEL2, C16_WSEL3][l]
        c16[:, woff:woff + NCH * g] = chunked(wm, NCH).astype(np.float16)
        boff = [C32_B1, C32_B2, C32_B3][l]
        c32[:, boff:boff + F] = np.broadcast_to(b, (P, F))
    for l, (Fin, F, g) in enumerate(LAYERS):
        b = np.asarray(inputs[f"b{l+1}"], dtype=np.float32)
        c32[0:F, C32_BT + l] = b
    c16[:, C16_IDT:C16_IDT + 128] = np.eye(P, dtype=np.float16)
    xt2 = x.T.reshape(4, P, 8, P).transpose(1, 2, 0, 3).reshape(P, 4 * N)
    c16[:, C16_XT:C16_XT + 4 * N] = xt2.astype(np.float16)
    c32[:, C32_IDENT:C32_IDENT + 128] = np.eye(P, dtype=np.float32)

    # adj i-major fp16: adjt[p, i*8 + kc] = adj[i, kc*128+p]
    adjT = (adj.T > 0).astype(np.float32)      # [j, i]
    adjt = adjT.reshape(NCH, P, N).transpose(1, 2, 0).reshape(P, N * NCH)
    adjt = adjt.astype(np.float16)

    l2w = np.asarray(inputs["l2w"], dtype=np.float32)   # [6,128,256]
    l2b = np.asarray(inputs["l2b"], dtype=np.float32)
    l3w = np.asarray(inputs["l3w"], dtype=np.float32)   # [6,1,128]
    l3b = np.asarray(inputs["l3b"], dtype=np.float32)
    c32r = np.zeros((6, 257), dtype=np.float32)
    c32r[:, 0:128] = l2b
    c32r[:, 128:256] = l3w[:, 0, :]
    c32r[:, 256] = l3b.reshape(-1)

    l1w_flat = np.asarray(inputs["l1w"], dtype=np.float32).reshape(1536, 65536)
    l1b_flat = np.asarray(inputs["l1b"], dtype=np.float32).reshape(1536)
    l1w_q = (l1w_flat * SA).astype(f8)

    # t1 index r = h*256 + t contracts only into head h: block-diagonal
    l2big = np.zeros((1536, 768), dtype=np.float32)
    for h in range(6):
        l2big[ts(h, 256), ts(h, 128)] = l2w[h].T        # [256,128]

    # matvec step V eats xf cols {c0, c0+1, c0+16, c0+17},
    # c0 = 32*(V//8) + 2*(V%8); A position V*4 + s*2 + j <- col c0+16s+j
    V = np.arange(KCH // 4)
    c0 = 32 * (V // 8) + 2 * (V % 8)
    perm = np.stack([c0, c0 + 1, c0 + 16, c0 + 17], axis=1).reshape(-1)

    common = dict(C16=c16, ADJT=adjt, C32=c32, C32R=c32r)
    in_maps = []
    for c in range(NCORES):
        rows = l1w_q[ts(c, RSHARD)]                     # [192, 65536]
        # xf col t = m*64 + f holds k-tile {(m*128+p)*64 + f : p}
        A = rows.reshape(RSHARD, 8, 128, 64)            # [r, m, p, f]
        A = A.transpose(2, 1, 3, 0).reshape(P, KCH, RSHARD)
        A = A[:, perm, :]
        sub = l2big[ts(c, RSHARD)]                      # [192, 768]
        l2sa = sub[0:128]
        l2sb = np.zeros((P, 768), dtype=np.float32)
        l2sb[0:64] = sub[128:192]
        lbv = np.zeros((P, 2), dtype=np.float32)
        lbv[:, 0] = l1b_flat[c * RSHARD:c * RSHARD + 128]
        lbv[0:64, 1] = l1b_flat[c * RSHARD + 128:(c + 1) * RSHARD]
        m = dict(common)
        m["L2B8"] = (l2b.reshape(1, 768) / NCORES).astype(np.float32)
        m["A8"] = np.ascontiguousarray(A)
        m["L2S"] = np.concatenate([l2sa, l2sb], axis=1).astype(np.float16)
        m["LB"] = lbv
        in_maps.append(m)
    return in_maps


def _ensure_ntff_hook():
    """Register the axon NTFF profile hook (the image's antenv lacks
    axon_hooks; supply it in sys.modules so bass_utils can trace)."""
    try:
        import types

        import antenv
        if "antenv.axon_hooks" not in sys.modules:
            mod = types.ModuleType("antenv.axon_hooks")
            mod._hook = None

            def _set(h, _m=mod):
                _m._hook = h

            def _get(_m=mod):
                return _m._hook

            mod.set_axon_ntff_profile_hook = _set
            mod.get_axon_ntff_profile_hook = _get
            sys.modules["antenv.axon_hooks"] = mod
            antenv.axon_hooks = mod
        from antenv.axon_hooks import (get_axon_ntff_profile_hook,
                                       set_axon_ntff_profile_hook)
        if get_axon_ntff_profile_hook() is None:
            from trn_agent_boot.trn_boot import _ntff_profile_via_ctypes
            set_axon_ntff_profile_hook(
                _ntff_profile_via_ctypes("/opt/axon/libaxon_pjrt.so"))
        return True
    except Exception as e:  # pragma: no cover - profiling is best-effort
        print(f"ntff hook unavailable: {e}", file=sys.stderr)
        return False


def kernel(**inputs) -> np.ndarray:
    if "nc" not in _CACHE:
        _CACHE["nc"] = _build()
    nc = _CACHE["nc"]
    in_maps = _prep_inputs(inputs)
    trace = bool(int(os.environ.get("BASS_KERNEL_TRACE", "0")))
    if trace:
        trace = _ensure_ntff_hook()
    res = run_bass_kernel_spmd(nc, in_maps, list(range(NCORES)), trace=trace)
    _CACHE["last_results"] = res
    return np.asarray(res.results[0]["out"],
                      dtype=np.float32).reshape(6, 1)



# revision 5
# speedup vs baseline: 1.0478x; 1.0478x over previous
"""Trainium2 Bass kernel for nn_GAT_12232066859439.

3-layer GAT + 6-head MLP readout, ~133-153us HW (baseline 192-205us).

  - GAT replicated on all 8 cores. The scrambled-view attention collapses to
    att[i,j] = adj[i,j]*c[j, i//F] / rowsum with c = exp(lrelu(e1[j] +
    e2T[j%F, u])); e1 rides as an extra column of the projection matmul
    (precomposed W @ a[:F]); e2's F=64 partition-dup is a second matmul
    group at partition offset 64 (a sbuf-sbuf DMA would queue ~11us behind
    the bulk A8 stream).
    Layer 1 (F=128): haug f-major, M2_u = haug * c with inner-stride-1 APs.
    Layers 2/3 (F=64): w = adj*c in the i-major adj layout (elementwise,
    fast mode); per-kc weight slices are single-stride APs.
    Per layer: all scale-muls, then all matmul groups, then all epilogues —
    interleaved emission head-of-line blocks the FIFO engine queues.
    nps psum is copied to SBUF immediately so matmul groups never throttle
    on epilogue drain; the h-bias is applied per-partition after the
    transpose (one DVE op fewer per chunk).
  - The 100MB l1 matvec is sharded 192 rows/core, stored fp8e4 (x32) with
    fp8 xf (x256), DoubleRow perf mode (weights need the 3D [Ki,2,dim] AP
    with Ko step % 16 == 0 and a 16-bit step field: xf pairs sit 16 cols
    apart, A8 tile order is host-permuted to keep the moving s-stride at
    384). Layer-3 output writes straight into xf (no transpose); the matvec
    runs as one warm PE stream right behind the layer-3 matmuls (163ns per
    out-[2,384] step, the DoubleRow floor).
  - Each core computes its partial l2 head contraction (block-diagonal
    [192,768] slice of l2w); one AllReduce of [1,768] replaces the old
    AllGather + on-core l2; sigmoid/l3 tail runs as [6,128] on all cores.
  - All constants ride in a few packed fan-out DMAs; xT is kc-major so the
    first hproj only needs the first 1028 const columns.
  - A dummy AllGather gated on the last xf chunk warms the CC and absorbs
    the cross-core launch skew during the matvec, so the real AllReduce
    starts warm (~1us trigger-to-mesh instead of ~11.5us cold) with a
    short rendezvous; l2b/8 is folded into each core's z2 partials.
  Typical HW time 124-138us; residual variance is launch-skew absorbed at
  the dummy rendezvous plus ~9us of end-of-kernel semaphore teardown.
"""
import os
import sys

sys.path.insert(0, "/opt/trn_rl_repo")

import numpy as np

import concourse.bacc as bacc
import concourse.bass as bass
import concourse.tile as tile
from concourse import mybir
from concourse.bass_utils import run_bass_kernel_spmd

F32 = mybir.dt.float32
F16 = mybir.dt.float16
F8 = mybir.dt.float8e4
AF = mybir.ActivationFunctionType
ALU = mybir.AluOpType
PM = mybir.MatmulPerfMode

P = 128
N = 1024
NCORES = 8
NCH = N // P
LAYERS = [(512, 128, 8), (128, 64, 16), (64, 64, 16)]
RSHARD = 1536 // NCORES      # 192 l1 rows per core
KCH = 65536 // P             # 512 k-tiles for the matvec
SA = 32.0                    # A fp8 scale
SX = 256.0                   # xf fp8 scale
WARM_COLLECTIVE = int(os.environ.get("GAT_WARM_COLL", "0"))

# C16 packed fp16 const layout (columns); Waug1+xT first so the first
# hproj can start as soon as the first const DMA lands
C16_WAUG1 = 0                # [P, 4*129]
C16_XT = 516                 # [P, 4*1024]
C16_WAUG2 = 4612             # [P, 65]
C16_WAUG3 = 4677             # [64, 65]
C16_WSEL1 = 4742             # [P, 8*8]
C16_WSEL2 = 4806             # [P, 8*16]
C16_WSEL3 = 4934             # [P, 8*16]
C16_IDT = 5062               # [P, 128] fp16 identity
C16_COLS = 5062 + 128

# C32 packed fp32 const layout
C32_IDENT = 0                # [P, 128]
C32_B1 = 128                 # [P, 128]
C32_B2 = 256                 # [P, 64]
C32_B3 = 320                 # [P, 64]
C32_BT = 384                 # [P, 3] per-feature biases (post-transpose)
C32_COLS = 388

_CACHE = {}


def ts(i, n):
    return slice(i * n, (i + 1) * n)


def _build():
    nc = bacc.Bacc("TRN2", target_bir_lowering=False, debug=False,
                   num_devices=NCORES)

    c16_d = nc.dram_tensor("C16", [P, C16_COLS], F16, kind="ExternalInput")
    adj_d = nc.dram_tensor("ADJT", [P, N * NCH], F16, kind="ExternalInput")
    c32_d = nc.dram_tensor("C32", [P, C32_COLS], F32, kind="ExternalInput")
    c32r_d = nc.dram_tensor("C32R", [6, 257], F32, kind="ExternalInput")
    a8_d = nc.dram_tensor("A8", [P, KCH, RSHARD], F8, kind="ExternalInput")
    l2s_d = nc.dram_tensor("L2S", [P, 2 * 768], F16, kind="ExternalInput")
    lb_d = nc.dram_tensor("LB", [P, 2], F32, kind="ExternalInput")
    l2b8_d = nc.dram_tensor("L2B8", [1, 768], F32, kind="ExternalInput")
    out_d = nc.dram_tensor("out", [6, 1], F32, kind="ExternalOutput")

    with tile.TileContext(nc) as tc:
        with tc.tile_pool(name="const", bufs=1) as const, \
             tc.tile_pool(name="work", bufs=1) as work, \
             tc.tile_pool(name="pmv", bufs=1, space="PSUM") as pmv, \
             tc.tile_pool(name="dram", bufs=1, space="DRAM") as dram:

            if WARM_COLLECTIVE:
                # 1 = full-group warmup (syncs cores, can stall via sem
                # reuse); 2 = per-core self-groups (no rendezvous)
                groups = ([list(range(NCORES))] if WARM_COLLECTIVE == 1
                          else [[c] for c in range(NCORES)])
                wsb = const.tile([1, 8], F32, name="wsb")
                nc.vector.memset(wsb[:], 1.0)
                w_in = dram.tile([1, 8], F32, name="w_in")
                w_out = dram.tile([1, 8], F32, name="w_out",
                                  addr_space="Shared")
                nc.sync.dma_start(w_in[:], wsb[:])
                nc.gpsimd.collective_compute(
                    "AllReduce", ALU.add,
                    replica_groups=groups,
                    ins=[w_in.opt()], outs=[w_out.opt()])

            # ---- packed constant loads (each dma_start fans out over all
            # DMA queues; trigger issue on sync costs ~0.65us apiece) ----
            c16 = const.tile([P, C16_COLS], F16, name="c16")
            nc.sync.dma_start(c16[:, 0:1028], c16_d[:, 0:1028])
            nc.sync.dma_start(c16[:, 1028:C16_WAUG2], c16_d[:, 1028:C16_WAUG2])
            nc.sync.dma_start(c16[:, C16_WAUG2:], c16_d[:, C16_WAUG2:])
            c32 = const.tile([P, C32_COLS], F32, name="c32")
            nc.sync.dma_start(c32[:], c32_d[:])
            adjt = const.tile([P, N * NCH], F16, name="adjt")
            for i in range(2):
                nc.sync.dma_start(adjt[:, ts(i, 4 * N)],
                                  adj_d[:, ts(i, 4 * N)])
            c32r = const.tile([6, 257], F32, name="c32r")
            nc.sync.dma_start(c32r[:], c32r_d[:])
            l2s = const.tile([P, 2 * 768], F16, name="l2s")
            nc.sync.dma_start(l2s[:], l2s_d[:])
            lb = const.tile([P, 2], F32, name="lb")
            nc.sync.dma_start(lb[:], lb_d[:])
            l2b8 = const.tile([1, 768], F32, name="l2b8")
            nc.sync.dma_start(l2b8[:], l2b8_d[:])
            a8 = const.tile([P, KCH, RSHARD], F8, name="a8")
            for s in range(4):
                nc.sync.dma_start(a8[:, ts(s, 128), :], a8_d[:, ts(s, 128), :])

            ident = c32[:, C32_IDENT:C32_IDENT + 128]
            idt = c16[:, C16_IDT:C16_IDT + 128]
            waug = [c16[:, C16_WAUG1:C16_WAUG1 + 516],
                    c16[:, C16_WAUG2:C16_WAUG2 + 65],
                    c16[0:64, C16_WAUG3:C16_WAUG3 + 65]]
            wsel = [c16[:, C16_WSEL1:C16_WSEL1 + 64],
                    c16[:, C16_WSEL2:C16_WSEL2 + 128],
                    c16[:, C16_WSEL3:C16_WSEL3 + 128]]
            bT = c32[:, C32_BT:C32_BT + 3]
            bb = [c32[:, C32_B1:C32_B1 + 128],
                  c32[:, C32_B2:C32_B2 + 64],
                  c32[:, C32_B3:C32_B3 + 64]]
            xT = c16[:, C16_XT:C16_XT + 4 * N]
            adjv = adjt.rearrange("p (i k) -> p i k", k=NCH)  # adj i-major

            xf3 = work.tile([P, KCH], F8, name="xf3")
            xfv = xf3.rearrange("p (q s w) -> p q s w", s=2, w=16)
            a8v = a8.rearrange("p (v s j) r -> p v s (j r)", s=2, j=2)
            t1ps = pmv.tile([2, 2 * RSHARD], F32, name="t1ps")
            hT = None

            # ---- GAT layers ----
            with tc.tile_pool(name="psg", bufs=1, space="PSUM") as psg:
                for l, (Fin, F, g) in enumerate(LAYERS):
                    nk = max(1, Fin // P)
                    kp = min(P, Fin)
                    FA = F + 1  # h features + ones column (kc-major blocks)

                    # layer 1 haug is f-major (fast M2 muls); layers 2/3 are
                    # kc-major (fast hproj copies, w rides on the adj side)
                    haug = work.tile([P, FA * NCH], F16, name=f"haug{l}")
                    hfv = haug.rearrange("p (f k) -> p f k", k=NCH)
                    e1 = work.tile([P, NCH], F32, name=f"e1_{l}")
                    if l == 0:
                        nc.vector.memset(haug[:, F * NCH:FA * NCH], 1.0)
                    else:
                        nc.vector.memset(
                            haug.rearrange("p (k f) -> p k f",
                                           f=FA)[:, :, F], 1.0)

                    for kc in range(NCH):
                        hp = psg.tile([P, 129], F32, name=f"hp{l}_{kc}",
                                      tag="ps129", bufs=5)
                        for ks in range(nk):
                            if l == 0:
                                lhsT = xT[:, kc * 512 + ks * P:
                                          kc * 512 + (ks + 1) * P]
                                rhs = waug[0][:, ts(ks, 129)]
                            else:
                                lhsT = hT[0:kp, ts(kc, P)]
                                rhs = waug[l]
                            nc.tensor.matmul(hp[:, 0:FA], lhsT, rhs,
                                             start=(ks == 0),
                                             stop=(ks == nk - 1))
                        if l == 0:
                            nc.vector.tensor_copy(hfv[:, 0:F, kc],
                                                  hp[:, 0:F])
                        else:
                            nc.scalar.activation(haug[:, kc * FA:kc * FA + F],
                                                 hp[:, 0:F], AF.Copy)
                        nc.scalar.activation(e1[:, kc:kc + 1], hp[:, F:F + 1],
                                             AF.Copy)

                    if l == 0:
                        # warm the CC early: dummy collective gated on e1 of
                        # layer 0 (~20us) so the CC stream finishes rendezvous
                        # + cold-start long before z2 is ready (~99us); the
                        # real AllGather then starts warm with no stream block
                        w8i = dram.tile([1, 8], F32, name="w8i")
                        nc.sync.dma_start(w8i[:], e1[0:1, 0:8])
                        w8o = dram.tile([8, 8], F32, name="w8o",
                                        addr_space="Shared")
                        nc.gpsimd.collective_compute(
                            "AllGather", ALU.bypass,
                            replica_groups=[list(range(NCORES))],
                            ins=[w8i.opt()], outs=[w8o.opt()])

                    # e2T[v, u] (v = j mod F); for F=64 write the upper
                    # partition half with a second matmul group (a sbuf-sbuf
                    # dup DMA would queue behind the bulk A8 stream)
                    e2p = psg.tile([P, 129], F32, name=f"e2p{l}", tag="ps129",
                                   bufs=5)
                    for m in range(NCH):
                        lhsT2 = (hfv[:, 0:F, m] if l == 0
                                 else haug[:, m * FA:m * FA + F])
                        nc.tensor.matmul(e2p[0:F, 0:g], lhsT2,
                                         wsel[l][:, ts(m, g)],
                                         start=(m == 0), stop=(m == NCH - 1))
                    if F == 64:
                        for m in range(NCH):
                            lhsT2 = haug[:, m * FA:m * FA + F]
                            nc.tensor.matmul(e2p[64:128, 0:g], lhsT2,
                                             wsel[l][:, ts(m, g)],
                                             start=(m == 0),
                                             stop=(m == NCH - 1))

                    # c[j, u] = exp(lrelu(e1[j] + e2T[j%F, u])), u-major
                    s_scr = work.tile([P, g * NCH], F32, name=f"sscr{l}")
                    sv = s_scr.rearrange("p (u k) -> p u k", k=NCH)
                    nc.vector.tensor_add(
                        sv, e1[:].broadcast_to([P, NCH, g]).rearrange(
                            "p k u -> p u k"),
                        e2p[:, 0:g].broadcast_to([P, g, NCH]))
                    nc.vector.scalar_tensor_tensor(s_scr[:], s_scr[:], 0.2,
                                                   s_scr[:], ALU.mult,
                                                   ALU.max)
                    e_all = work.tile([P, g * NCH], F16, name=f"eall{l}")
                    nc.scalar.activation(e_all[:], s_scr[:], AF.Exp)

                    # phase A: all scale-muls, fast-mode APs (inner stride 1
                    # on every operand), emitted before any epilogue ops so
                    # the DVE queue never head-of-line blocks
                    sc = []
                    for q in range(NCH):
                        M2 = work.tile([P, 1032], F16,
                                       name=f"M2_{l}_{q}", tag="M2", bufs=8)
                        if l == 0:
                            # M2_u[p, f*8+k] = haug[p, f, k] * c[p-as-j, u]
                            nc.vector.tensor_mul(
                                M2[:, 0:FA * NCH].rearrange(
                                    "p (f k) -> p f k", k=NCH),
                                hfv,
                                e_all[:, ts(q, NCH)].broadcast_to(
                                    [P, NCH, FA]).rearrange("p k f -> p f k"))
                        else:
                            # w2[p, i_loc*8 + k] = adj[i, j]*c[j, 2q+h(i)]
                            # (same i-major layout as adjt; i_loc = h*64+i2)
                            nc.vector.tensor_mul(
                                M2[:, 0:NCH * P].rearrange(
                                    "p (h i k) -> p h i k", h=2, k=NCH),
                                adjt[:, q * NCH * P:(q + 1) * NCH * P]
                                .rearrange("p (h i k) -> p h i k",
                                           h=2, k=NCH),
                                e_all[:, ts(q, 16)].rearrange(
                                    "p (h k) -> p h k", k=NCH).broadcast_to(
                                    [P, 2, NCH, 64]).rearrange(
                                    "p h k i -> p h i k"))
                        sc.append(M2)

                    # phase B/C: matmul groups + per-chunk epilogues
                    if l == 1:
                        hTn = work.tile([P, N], F16, name="hT1")
                    elif l == 0:
                        hT = work.tile([P, N], F16, name="hT0")
                    # phase B: every att matmul group back-to-back on the
                    # PE queue (epilogue PE ops would head-of-line block
                    # later groups otherwise, and the PE never warms)
                    npss = []
                    for q in range(NCH):
                        nps = psg.tile([P, 129], F32, name=f"nps{l}_{q}",
                                       tag="ps129", bufs=5)
                        M2v = (None if l == 0 else
                               sc[q][:, 0:NCH * P].rearrange(
                                   "p (i k) -> p i k", k=NCH))
                        for kc in range(NCH):
                            if l == 0:
                                nc.tensor.matmul(
                                    nps[:, 0:FA], adjv[:, ts(q, P), kc],
                                    sc[q][:, 0:FA * NCH].rearrange(
                                        "p (f k) -> p f k",
                                        k=NCH)[:, :, kc],
                                    start=(kc == 0), stop=(kc == NCH - 1))
                            else:
                                nc.tensor.matmul(
                                    nps[:, 0:FA], M2v[:, :, kc],
                                    haug[:, ts(kc, FA)],
                                    start=(kc == 0), stop=(kc == NCH - 1))
                        # free the psum slot right away so later matmul
                        # groups never throttle on epilogue drain
                        nsb = work.tile([P, 132], F32, name=f"nsb{l}_{q}",
                                        tag="nsb", bufs=8)
                        nc.vector.tensor_copy(nsb[:, 0:FA], nps[:, 0:FA])
                        npss.append(nsb)
                    # phase C: epilogues (from SBUF copies)
                    for q in range(NCH):
                        nps = npss[q]
                        rd = work.tile([P, 1], F32, name=f"rd{l}_{q}",
                                       tag="rd", bufs=4)
                        nc.vector.reciprocal(rd[:], nps[:, F:F + 1])
                        y = work.tile([P, P], F16, name=f"y{l}_{q}",
                                      tag="y", bufs=4)
                        nc.scalar.activation(y[:, 0:F], nps[:, 0:F], AF.Relu,
                                             scale=rd[:])
                        if l == 0:
                            # bias rides per-partition after the transpose
                            tp = psg.tile([P, P], F16, name=f"tp{l}_{q}",
                                          tag="tp", bufs=2)
                            nc.tensor.transpose(tp[:], y[:], idt)
                            nc.scalar.activation(hT[:, ts(q, P)], tp[:],
                                                 AF.Relu, bias=bT[:, 0:1])
                        elif l == 1:
                            tp = psg.tile([P, P], F16, name=f"tp{l}_{q}",
                                          tag="tp", bufs=2)
                            nc.tensor.transpose(tp[0:64, :], y[:, 0:64],
                                                idt)
                            nc.scalar.activation(hTn[0:64, ts(q, P)],
                                                 tp[0:64, :], AF.Relu,
                                                 bias=bT[0:64, 1:2])
                        else:
                            hn = work.tile([P, P], F16, name=f"hn{l}_{q}",
                                           tag="hn", bufs=4)
                            nc.vector.tensor_add(hn[:, 0:F], y[:, 0:F],
                                                 bb[l])
                            nc.scalar.activation(xf3[:, ts(q, 64)],
                                                 hn[:, 0:64], AF.Relu,
                                                 scale=SX)
                    if l == 2:
                        # matvec after all epilogues are queued: step V eats
                        # xf cols {c0, c0+1, c0+16, c0+17} with
                        # c0 = 32*(V//8) + 2*(V%8); runs as one warm PE
                        # stream right behind the layer-3 matmuls
                        for V in range(128):
                            nc.tensor.matmul(
                                t1ps[:],
                                xfv[:, V // 8, :,
                                    2 * (V % 8):2 * (V % 8) + 2],
                                a8v[:, V, :, :],
                                start=(V == 0), stop=(V == 127),
                                perf_mode=PM.DoubleRow)
                    if l == 1:
                        hT = hTn

            # hoist activation-table loads into the matvec window
            dact = work.tile([1, 2], F32, name="dact")
            nc.scalar.activation(dact[:, 0:1], xf3[:1, 0:1], AF.Sigmoid)
            nc.scalar.activation(dact[:, 1:2], xf3[:1, 0:1], AF.Relu,
                                 bias=lb[0:1, 0:1])

            with tc.tile_pool(name="pst", bufs=1, space="PSUM") as pst:
                # t1 = relu(z/(SA*SX) + b): transpose both psum rows in
                # column pairs (no cross-partition DMA), add on partitions
                t1c = work.tile([2, 2 * RSHARD], F32, name="t1c")
                nc.vector.tensor_copy(t1c[:], t1ps[:])
                tps = []
                for i, (c0, c1) in enumerate(((0, 128), (192, 320),
                                              (128, 192), (320, 384))):
                    tt = pst.tile([P, 2], F32, name=f"tt{i}", tag="tt",
                                  bufs=4)
                    nc.tensor.transpose(tt[0:c1 - c0, :], t1c[:, c0:c1],
                                        ident[0:2, 0:2])
                    tps.append(tt)
                sb1 = work.tile([P, 2], F32, name="sb1")
                nc.scalar.activation(sb1[:, 0:1], tps[1][:, 1:2], AF.Copy)
                nc.scalar.activation(sb1[0:64, 1:2], tps[3][0:64, 1:2],
                                     AF.Copy)
                za = work.tile([P, 1], F32, name="za")
                nc.vector.tensor_add(za[:], tps[0][:, 0:1], sb1[:, 0:1])
                zb = work.tile([64, 1], F32, name="zb")
                nc.vector.tensor_add(zb[:], tps[2][0:64, 0:1],
                                     sb1[0:64, 1:2])
                t1sa = work.tile([P, 1], F16, name="t1sa")
                nc.scalar.activation(t1sa[:], za[:], AF.Relu,
                                     scale=1.0 / (SA * SX), bias=lb[:, 0:1])
                t1sb = work.tile([64, 1], F16, name="t1sb")
                nc.scalar.activation(t1sb[:], zb[:], AF.Relu,
                                     scale=1.0 / (SA * SX), bias=lb[0:64, 1:2])

                z2 = work.tile([1, 768], F32, name="z2")
                for half in range(2):
                    ps2 = pst.tile([1, 384], F32, name=f"ps2_{half}",
                                   tag="ps2", bufs=2)
                    nc.tensor.matmul(ps2[:], t1sa[:],
                                     l2s[:, half * 384:half * 384 + 384],
                                     start=True, stop=False)
                    nc.tensor.matmul(
                        ps2[:], t1sb[:],
                        l2s[0:64, 768 + half * 384:768 + half * 384 + 384],
                        start=False, stop=True)
                    nc.vector.tensor_add(z2[:, ts(half, 384)], ps2[:],
                                         l2b8[:, ts(half, 384)])

                # AllGather (ring: n-1 hops, vs 2(n-1) for AllReduce) +
                # local tree-sum of the 8 z2 partials
                rr_in = dram.tile([1, 768], F32, name="rr_in")
                rr_out = dram.tile([8, 768], F32, name="rr_out",
                                   addr_space="Shared")
                nc.sync.dma_start(rr_in[:], z2[:])
                nc.gpsimd.collective_compute(
                    "AllGather", ALU.bypass,
                    replica_groups=[list(range(NCORES))],
                    ins=[rr_in.opt()], outs=[rr_out.opt()])

                # ---- tail on [6, 128]: sigmoid(z+l2b), l3w.t2 + l3b ----
                zz8 = work.tile([6, 8 * P], F32, name="zz8")
                nc.sync.dma_start(
                    zz8.rearrange("h (c o) -> h c o", o=P),
                    rr_out.rearrange("c (h o) -> h c o", o=P))
                g1 = work.tile([6, 4 * P], F32, name="g1")
                nc.vector.tensor_add(g1[:], zz8[:, 0:4 * P],
                                     zz8[:, 4 * P:8 * P])
                g2 = work.tile([6, 2 * P], F32, name="g2")
                nc.vector.tensor_add(g2[:], g1[:, 0:2 * P],
                                     g1[:, 2 * P:4 * P])
                zz = work.tile([6, P], F32, name="zz")
                nc.vector.tensor_add(zz[:], g2[:, 0:P], g2[:, P:2 * P])
                t2 = work.tile([6, P], F32, name="t2")
                nc.scalar.activation(t2[:], zz[:], AF.Sigmoid)
                p3 = work.tile([6, P], F32, name="p3")
                nc.vector.tensor_mul(p3[:], t2[:], c32r[:, 128:256])
                o6 = work.tile([6, 1], F32, name="o6")
                nc.vector.reduce_sum(o6[:], p3[:], axis=mybir.AxisListType.X)
                oo = work.tile([6, 1], F32, name="oo")
                nc.vector.tensor_add(oo[:], o6[:], c32r[:, 256:257])
                nc.sync.dma_start(out_d[:], oo[:])

    nc.compile()
    return nc


def _prep_inputs(inputs):
    f8 = mybir.dt.np(F8)
    x = np.asarray(inputs["x"], dtype=np.float32)
    adj = np.asarray(inputs["adj"])

    def chunked(arr, nch):
        # [nch*P, C] -> [P, nch*C] with block kc at cols [kc*C:(kc+1)*C]
        c = arr.shape[1]
        return arr.reshape(nch, P, c).transpose(1, 0, 2).reshape(P, nch * c)

    c16 = np.zeros((P, C16_COLS), dtype=np.float16)
    c32 = np.zeros((P, C32_COLS), dtype=np.float32)
    for l, (Fin, F, g) in enumerate(LAYERS):
        W = np.asarray(inputs[f"W{l+1}"], dtype=np.float64)
        a = np.asarray(inputs[f"a{l+1}"], dtype=np.float64)
        b = np.asarray(inputs[f"b{l+1}"], dtype=np.float32)
        waug = np.concatenate([W, (W @ a[:F])[:, None]], axis=1)  # [Fin,F+1]
        off = [C16_WAUG1, C16_WAUG2, C16_WAUG3][l]
        if l == 0:
            c16[:, off:off + 516] = chunked(waug, 4).astype(np.float16)
        else:
            c16[0:Fin, off:off + F + 1] = waug.astype(np.float16)
        aS = a[F:]
        i = np.arange(N)
        wm = np.zeros((N, g), dtype=np.float64)
        wm[i, i % g] = aS[i // g]
        woff = [C16_WSEL1, C16_WSEL2, C16_WSEL3][l]
        c16[:, woff:woff + NCH * g] = chunked(wm, NCH).astype(np.float16)
        boff = [C32_B1, C32_B2, C32_B3][l]
        c32[:, boff:boff + F] = np.broadcast_to(b, (P, F))
    for l, (Fin, F, g) in enumerate(LAYERS):
        b = np.asarray(inputs[f"b{l+1}"], dtype=np.float32)
        c32[0:F, C32_BT + l] = b
    c16[:, C16_IDT:C16_IDT + 128] = np.eye(P, dtype=np.float16)
    xt2 = x.T.reshape(4, P, 8, P).transpose(1, 2, 0, 3).reshape(P, 4 * N)
    c16[:, C16_XT:C16_XT + 4 * N] = xt2.astype(np.float16)
    c32[:, C32_IDENT:C32_IDENT + 128] = np.eye(P, dtype=np.float32)

    # adj i-major fp16: adjt[p, i*8 + kc] = adj[i, kc*128+p]
    adjT = (adj.T > 0).astype(np.float32)      # [j, i]
    adjt = adjT.reshape(NCH, P, N).transpose(1, 2, 0).reshape(P, N * NCH)
    adjt = adjt.astype(np.float16)

    l2w = np.asarray(inputs["l2w"], dtype=np.float32)   # [6,128,256]
    l2b = np.asarray(inputs["l2b"], dtype=np.float32)
    l3w = np.asarray(inputs["l3w"], dtype=np.float32)   # [6,1,128]
    l3b = np.asarray(inputs["l3b"], dtype=np.float32)
    c32r = np.zeros((6, 257), dtype=np.float32)
    c32r[:, 0:128] = l2b
    c32r[:, 128:256] = l3w[:, 0, :]
    c32r[:, 256] = l3b.reshape(-1)

    l1w_flat = np.asarray(inputs["l1w"], dtype=np.float32).reshape(1536, 65536)
    l1b_flat = np.asarray(inputs["l1b"], dtype=np.float32).reshape(1536)
    l1w_q = (l1w_flat * SA).astype(f8)

    # t1 index r = h*256 + t contracts only into head h: block-diagonal
    l2big = np.zeros((1536, 768), dtype=np.float32)
    for h in range(6):
        l2big[ts(h, 256), ts(h, 128)] = l2w[h].T        # [256,128]

    # matvec step V eats xf cols {c0, c0+1, c0+16, c0+17},
    # c0 = 32*(V//8) + 2*(V%8); A position V*4 + s*2 + j <- col c0+16s+j
    V = np.arange(KCH // 4)
    c0 = 32 * (V // 8) + 2 * (V % 8)
    perm = np.stack([c0, c0 + 1, c0 + 16, c0 + 17], axis=1).reshape(-1)

    common = dict(C16=c16, ADJT=adjt, C32=c32, C32R=c32r)
    in_maps = []
    for c in range(NCORES):
        rows = l1w_q[ts(c, RSHARD)]                     # [192, 65536]
        # xf col t = m*64 + f holds k-tile {(m*128+p)*64 + f : p}
        A = rows.reshape(RSHARD, 8, 128, 64)            # [r, m, p, f]
        A = A.transpose(2, 1, 3, 0).reshape(P, KCH, RSHARD)
        A = A[:, perm, :]
        sub = l2big[ts(c, RSHARD)]                      # [192, 768]
        l2sa = sub[0:128]
        l2sb = np.zeros((P, 768), dtype=np.float32)
        l2sb[0:64] = sub[128:192]
        lbv = np.zeros((P, 2), dtype=np.float32)
        lbv[:, 0] = l1b_flat[c * RSHARD:c * RSHARD + 128]
        lbv[0:64, 1] = l1b_flat[c * RSHARD + 128:(c + 1) * RSHARD]
        m = dict(common)
        m["L2B8"] = (l2b.reshape(1, 768) / NCORES).astype(np.float32)
        m["A8"] = np.ascontiguousarray(A)
        m["L2S"] = np.concatenate([l2sa, l2sb], axis=1).astype(np.float16)
        m["LB"] = lbv
        in_maps.append(m)
    return in_maps


def _ensure_ntff_hook():
    """Register the axon NTFF profile hook (the image's antenv lacks
    axon_hooks; supply it in sys.modules so bass_utils can trace)."""
    try:
        import types

        import antenv
        if "antenv.axon_hooks" not in sys.modules:
            mod = types.ModuleType("antenv.axon_hooks")
            mod._hook = None

            def _set(h, _m=mod):
                _m._hook = h

            def _get(_m=mod):
                return _m._hook

            mod.set_axon_ntff_profile_hook = _set
            mod.get_axon_ntff_profile_hook = _get
            sys.modules["antenv.axon_hooks"] = mod
            antenv.axon_hooks = mod
        from antenv.axon_hooks import (get_axon_ntff_profile_hook,
                                       set_axon_ntff_profile_hook)
        if get_axon_ntff_profile_hook() is None:
            from trn_agent_boot.trn_boot import _ntff_profile_via_ctypes
            set_axon_ntff_profile_hook(
                _ntff_profile_via_ctypes("/opt/axon/libaxon_pjrt.so"))
        return True
    except Exception as e:  # pragma: no cover - profiling is best-effort
        print(f"ntff hook unavailable: {e}", file=sys.stderr)
        return False


def kernel(**inputs) -> np.ndarray:
    if "nc" not in _CACHE:
        _CACHE["nc"] = _build()
    nc = _CACHE["nc"]
    in_maps = _prep_inputs(inputs)
    trace = bool(int(os.environ.get("BASS_KERNEL_TRACE", "0")))
    if trace:
        trace = _ensure_ntff_hook()
    res = run_bass_kernel_spmd(nc, in_maps, list(range(NCORES)), trace=trace)
    _CACHE["last_results"] = res
    return np.asarray(res.results[0]["out"],
                      dtype=np.float32).reshape(6, 1)

